# revision 1
# baseline (speedup 1.0000x reference)
"""Trainium2 Bass kernel for the Correlation module.

reference:
    affinities = einsum('lnd,ond->lon', x, upfold) / sqrt(d)   # [L,O,N]
    features   = einsum('lon,ond->lnd', sigmoid(affinities)-0.5, upfold)

Math used here: sigmoid(a)-0.5 = 0.5*tanh(a/2), so with s = 1/sqrt(64):
    W^T = tanh(A^T / 16)            (A = x @ upfold^T per n)
    F   = 0.5 * (W @ upfold)        (0.5 folded into the PSUM->SBUF copy)

All matmul operands are bf16 (inputs cast on load). All transposes run
on the DMA engines (X-bar dma transpose, 2-byte dtype) so the PE queue
contains nothing but the two matmul streams:
  mm1 (A^T = U X^T, contract d=64): PE row tiling packs the two K=64
      matmuls of a pair at tile_position (0,0)/(64,0) -> concurrent.
  mm2 (F^T = u^T W^T, contract o): stationary is u in NATURAL layout
      ([o,d], M=64); PE column tiling packs the two M=64 matmuls at
      tile_position (0,0)/(0,64) -> concurrent, no zero padding.
The tanh runs on the Scalar engine (the only engine with activation
tables) at ~(N+352)/1.2 ns per [128,N] tile; with 64 x N=1024 tiles per
core it is the ~69us pipelined throughput floor the rest of the
pipeline hides under. PSUM: 3 'at' bufs (6 banks) + ft (2 banks) so
mm1 of the next oc never waits on the serial tanh chain.

Every staging tile uses a per-pair tag: tags map to semaphores, and
reusing a tag across pairs creates serial DMA/sem chains plus slot-wait
instructions that head-of-line-block the in-order engine queues.
Pair 0's transposes issue on the scalar HWDGE (idle until the first
tanh) to shorten the startup chain.

Sharding: data-parallel over N across 8 cores (8 n per core), processed
as 4 pairs of n. Self-contained: hardcodes shapes.
"""

import numpy as np

L, N, D, O = 1024, 64, 64, 1024
NCORES = 8
NLOC = N // NCORES   # 8 n per core
NPAIRS = NLOC // 2   # 4 pairs

_CACHE = {}


def _build_program():
    import concourse.mybir as mybir
    import concourse.tile as tile
    from concourse import bacc

    f32 = mybir.dt.float32
    bf16 = mybir.dt.bfloat16
    TANH = mybir.ActivationFunctionType.Tanh

    nc = bacc.Bacc(
        "TRN2", target_bir_lowering=False, debug=False, num_devices=NCORES
    )
    x_ap = nc.dram_tensor("x", [L, NLOC, D], f32, kind="ExternalInput").ap()
    u_ap = nc.dram_tensor("upfold", [O, NLOC, D], f32, kind="ExternalInput").ap()
    o_ap = nc.dram_tensor("out", [L, NLOC, D], f32, kind="ExternalOutput").ap()

    with tile.TileContext(nc) as tc:
        with (
            tc.tile_pool(name="io", bufs=2) as iop,
            tc.tile_pool(name="xc", bufs=2) as xcp,
            tc.tile_pool(name="ub", bufs=2) as ubp,
            tc.tile_pool(name="tsp", bufs=2) as tsp,
            tc.tile_pool(name="wt", bufs=3) as wtp,
            tc.tile_pool(name="fsb", bufs=2) as fsbp,
            tc.tile_pool(name="ost", bufs=2) as ostp,
            tc.tile_pool(name="atps", bufs=3, space="PSUM") as atps,
            tc.tile_pool(name="ftps", bufs=1, space="PSUM") as ftps,
        ):
            loaded = {}
            staged = {}

            def load_pair(p):
                """Half-granular f32 DMAs so casts/transposes start early."""
                n0 = 2 * p
                tiles = []
                for src_ap, tagb in ((x_ap, "xp"), (u_ap, "up")):
                    full = src_ap[:, n0 : n0 + 2, :].rearrange(
                        "(lc q) n d -> q lc (n d)", q=128
                    )
                    t = iop.tile([128, 8, 128], f32, tag=tagb)
                    nc.sync.dma_start(t[:], full[:])
                    tiles.append(t)
                staged[p] = tiles

            def prep_pair(p):
                """Cast to bf16; build XT/UT ([d-pair, l/o]) via X-bar DMA
                transposes (out[r, 128k + q] = src[q, (k, r)]). The bf16
                natural-layout u tile doubles as mm2's stationary. Pair 0
                transposes on the pre-tanh idle scalar HWDGE."""
                xp_, up_ = staged.pop(p)
                xbt = xcp.tile([128, 8, 128], bf16, tag="xb")
                nc.vector.tensor_copy(xbt[:], xp_[:])
                ubt = ubp.tile([128, 8, 128], bf16, tag="ub")
                nc.vector.tensor_copy(ubt[:], up_[:])

                XT = tsp.tile([128, 1024], bf16, tag="XT")
                UT = tsp.tile([128, 1024], bf16, tag="UT")
                treng = nc.sync
                # One full-size transpose per tensor: DMA_TRANSPOSE holds
                # the issuing queue ~1.25us regardless of size.
                for dst, src_t in ((XT, xbt), (UT, ubt)):
                    treng.dma_start_transpose(
                        dst[:].rearrange("p (k q) -> p k q", q=128),
                        src_t[:].rearrange("q a b -> q (a b)"),
                    )
                loaded[p] = (XT, UT, ubt)

            def emit_out(p, fsb):
                # F^T -> F via X-bar DMA transpose, cast to f32, store.
                n0 = 2 * p
                dst = o_ap[:, n0 : n0 + 2, :].rearrange(
                    "(lc q) n d -> q lc (n d)", q=128
                )
                ostb = ostp.tile([128, 8, 128], bf16, tag="ostb")
                nc.sync.dma_start_transpose(ostb[:], fsb[:])
                ost = ostp.tile([128, 8, 128], f32, tag="ost")
                nc.vector.tensor_copy(ost[:], ostb[:])
                nc.sync.dma_start(dst[:, :, :], ost[:])

            def oc_loop(p, carry):
                XT, UT, ubt = loaded.pop(p)
                ft = ftps.tile([128, 1024], f32, tag="ft")

                def mm1_half(oc, ni, at):
                    rows = slice(64 * ni, 64 * (ni + 1))
                    for lh in range(2):
                        nc.tensor.matmul(
                            at[:, 512 * lh : 512 * (lh + 1)],
                            UT[rows, 128 * oc : 128 * (oc + 1)],
                            XT[rows, 512 * lh : 512 * (lh + 1)],
                            start=True,
                            stop=True,
                            tile_position=(64 * ni, 0),
                        )

                def mm2_half(oc, ni, w):
                    rows = slice(64 * ni, 64 * (ni + 1))
                    for lh in range(2):
                        nc.tensor.matmul(
                            ft[rows, 512 * lh : 512 * (lh + 1)],
                            ubt[:, oc, rows],
                            w[:, 512 * lh : 512 * (lh + 1)],
                            start=(oc == 0),
                            stop=(oc == 7),
                            tile_position=(0, 64 * ni),
                        )

                prev = None  # (oc, w0, w1) awaiting mm2
                pending = None
                for oc in range(8):
                    if oc == 1 and carry is not None:
                        pending = carry["fsb"]()
                    if oc == 3 and pending is not None:
                        emit_out(*pending)
                    if oc == 0 and p + 2 < NPAIRS:
                        load_pair(p + 2)
                    if oc == 2 and p + 2 < NPAIRS:
                        prep_pair(p + 2)
                    at0 = atps.tile([128, 1024], f32, tag="at")
                    at1 = atps.tile([128, 1024], f32, tag="at")
                    # Keep tiling partners adjacent in the PE queue: the two
                    # row-banded mm1 halves overlap (disjoint rows), then the
                    # two col-banded mm2 halves overlap (disjoint cols).
                    mm1_half(oc, 0, at0)
                    mm1_half(oc, 1, at1)
                    if prev is not None:
                        mm2_half(prev[0], 0, prev[1])
                        mm2_half(prev[0], 1, prev[2])
                    elif oc == 0 and carry is not None:
                        carry["mm2a"]()
                        carry["mm2b"]()
                    w0 = wtp.tile([128, 1024], bf16, tag="w0")
                    nc.scalar.activation(w0[:], at0[:], TANH, scale=1.0 / 16.0)
                    w1 = wtp.tile([128, 1024], bf16, tag="w1")
                    nc.scalar.activation(w1[:], at1[:], TANH, scale=1.0 / 16.0)
                    prev = (oc, w0, w1)

                def make_fsb():
                    fsb = fsbp.tile([128, 1024], bf16, name="fsb")
                    nc.vector.tensor_scalar_mul(fsb[:], ft[:], 0.5)
                    return (p, fsb)

                return {
                    "mm2a": lambda: mm2_half(7, 0, prev[1]),
                    "mm2b": lambda: mm2_half(7, 1, prev[2]),
                    "fsb": make_fsb,
                }

            load_pair(0)
            prep_pair(0)
            load_pair(1)
            prep_pair(1)
            carry = None
            for p in range(NPAIRS):
                carry = oc_loop(p, carry)
            carry["mm2a"]()
            carry["mm2b"]()
            emit_out(*carry["fsb"]())

    nc.compile()
    return nc


def _get_program():
    if "nc" not in _CACHE:
        _CACHE["nc"] = _build_program()
    return _CACHE["nc"]


def _make_in_maps(x, upfold):
    x = np.asarray(x, dtype=np.float32)
    upfold = np.asarray(upfold, dtype=np.float32)
    in_maps = []
    for c in range(NCORES):
        s = slice(NLOC * c, NLOC * (c + 1))
        in_maps.append(
            {
                "x": np.ascontiguousarray(x[:, s, :]),
                "upfold": np.ascontiguousarray(upfold[:, s, :]),
            }
        )
    return in_maps


def run_sharded(x, upfold, trace=False, **kwargs):
    """Run on all 8 cores; returns (full_output, BassKernelResults)."""
    from concourse.bass_utils import run_bass_kernel_spmd

    nc = _get_program()
    res = run_bass_kernel_spmd(
        nc, _make_in_maps(x, upfold), core_ids=list(range(NCORES)),
        trace=trace, **kwargs
    )
    out = np.concatenate([res.results[c]["out"] for c in range(NCORES)], axis=1)
    return out, res


def kernel(x, upfold):
    out, _ = run_sharded(x, upfold)
    return out

